# revision 26
# baseline (speedup 1.0000x reference)
"""Multi-head attention block (B=32,S=512,D=768,H=12) on 8 TRN2 NeuronCores.

Sharding: data-parallel over batch (4 batches/core), weights replicated,
no collectives. Host pre-transposes x and the weight matrices so the
device kernel is a pure matmul pipeline (no on-chip transposes):

  per core (4 batches), all matmul operands bf16 (host-converted), fp32
  accumulation in PSUM:
    yT[o,t]  = Wqkv xT for q,k rows (o on partitions -> ACT per-partition
             bias during the psum->sbuf copy)
    v[t,o]   natural, bias added from a partition-broadcast tile during the
             interleave copy; stored with an all-ones column per head:
             [v_h | 1] is the stationary operand of the av matmul, so row
             64 of the av output is the softmax denominator for free.
    per head: scoresT[s,t] = kT^T qT (K=64), exp on ACT ([128,1024] ops,
             scale folded, Exp table stays resident), av+sums in one
             matmul.  Normalization runs in waves of 3 heads: sums rows
             stack via DMA as 4x128 blocks at partition offsets 0/32/64,
             one DVE reciprocal per wave at free-dim 128, DMA hop to
             partition 0, gpsimd partition-broadcast, DVE bf16 multiply
             (odd heads partition-shifted into the packed avT via DMA).
    out[t,:] = avT^T WpT + combo; DVE adds combo during the psum->sbuf
             copy, halves DMA to DRAM as they complete.

Schedule: software-pipelined qkv(b) -> proj(b-1) -> attn(b) processed in
head PAIRS: the two heads of a yT chunk live on partitions 0:64/64:128,
so their K=64 score matmuls run CONCURRENTLY as 64x128 row tiles
((0,0)/(64,0)) when interleaved — ~2x scores throughput.  HAM warmup
matmuls on memset tiles run during the startup DMA window so real work
starts at 2.4 GHz, and the whole last batch normalizes straight from
psum (ACT reciprocal fast path) to shorten the tail chain.  x prefetch
and yT/v tiles of batch b+1 interleave into attn(b) slots.
PSUM pools are split: scores (2x2 banks), qkv/proj (2x1), av+v (2x1) so
the next batch's qkv chunks never wait on attention normalization
drains.  Startup DMAs land in consumption order (x three-way across
queues, qkv weight columns chunk-by-chunk).
"""

import sys

if "/opt/trn_rl_repo" not in sys.path:
    sys.path.insert(0, "/opt/trn_rl_repo")

from contextlib import ExitStack

import numpy as np

import concourse.tile as tile
from concourse import bacc, mybir
from concourse.bass import AP
from concourse.bass_utils import run_bass_kernel_spmd

B, S, D = 32, 512, 768
H, HD = 12, 64
SCALE = HD**-0.5
NCORES = 8
NB = B // NCORES  # batches per core
P = 128
TCH = S // P  # token chunks per batch
DCH = D // P  # d chunks
QKC = 2 * D // P  # o-chunks holding q,k
NHALF = D // 2  # 384: N-tile for v/proj matmuls
F32 = mybir.dt.float32
BF16 = mybir.dt.bfloat16
EXP = mybir.ActivationFunctionType.Exp

WAVE = 6


def _act_reciprocal(nc, out_ap, in_ap):
    """Raw ACT-table reciprocal (~1e-3 rel; softmax sums here are >> 2.5).
    The bass wrapper refuses Reciprocal for general use; emit InstActivation
    directly."""
    eng = nc.scalar
    ins_ = [eng.lower_ap(in_ap)]
    for arg in (0.0, 1.0, 0.0):  # bias, scale, alpha
        ins_.append(mybir.ImmediateValue(dtype=F32, value=arg))
    return eng.add_instruction(
        mybir.InstActivation(
            name=eng.bass.get_next_instruction_name(),
            func=mybir.ActivationFunctionType.Reciprocal,
            ins=ins_,
            outs=[eng.lower_ap(out_ap)],
        )
    )


def build_nc():
    nc = bacc.Bacc(None, target_bir_lowering=False, debug=False)
    xT = nc.declare_dram_parameter("xT", [NB, D, S], BF16, isOutput=False)
    wqkvT = nc.declare_dram_parameter("wqkvT", [D, 3 * D], BF16, isOutput=False)
    wpT = nc.declare_dram_parameter("wpT", [D, D], BF16, isOutput=False)
    bqkv = nc.declare_dram_parameter("bqkv", [3 * D], F32, isOutput=False)
    combo = nc.declare_dram_parameter("combo", [D], BF16, isOutput=False)
    bv16 = nc.declare_dram_parameter("bv16", [D], BF16, isOutput=False)
    out = nc.declare_dram_parameter("out", [NB, S, D], BF16, isOutput=True)

    with ExitStack() as ctx:
        tc = ctx.enter_context(tile.TileContext(nc))
        wp = ctx.enter_context(tc.tile_pool(name="weights", bufs=1))
        sb = ctx.enter_context(tc.tile_pool(name="work", bufs=1))
        ps = ctx.enter_context(tc.tile_pool(name="psum", bufs=1, space="PSUM"))

        # ---- the first yT chunk-pair's weight columns and x(0) gate the
        # PE start: interleave them across all three DMA queues ----
        x_engs = [nc.gpsimd, nc.scalar, nc.sync]
        wq_t = [
            wp.tile([P, 3 * D], BF16, name=f"wqkvT{d}", tag=f"wqkvT{d}")
            for d in range(DCH)
        ]
        xt0 = []
        for d in range(DCH):
            t = sb.tile([P, S], BF16, name=f"xT_b0_{d}", tag=f"xT{d}", bufs=2)
            x_engs[d % 3].dma_start(out=t, in_=xT[0, d * P : (d + 1) * P, :])
            xt0.append(t)
        for d in range(DCH):
            x_engs[d % 3].dma_start(
                out=wq_t[d][:, :P], in_=wqkvT[d * P : (d + 1) * P, :P]
            )
        for d in range(DCH):
            x_engs[d % 3].dma_start(
                out=wq_t[d][:, 6 * P : 7 * P],
                in_=wqkvT[d * P : (d + 1) * P, 6 * P : 7 * P],
            )
        # ---- HAM warmup: the first ~13.5us are DMA-bound (runtime
        # preamble + x/weight loads), during which the PE would idle cold
        # (K=4/8 = 1.2 GHz) and only reach 2.4 GHz ~4.7us into real work.
        # Stream dummy matmuls on memset tiles so the clock gate opens
        # during the DMA wait: ~8 cold MMs (3.4us) flip HAM, the rest
        # keep it open until real operands land. ----
        dum_w = wp.tile([P, 4], BF16, name="dum_w", tag="dum_w")
        nc.vector.memset(dum_w, 0.0)
        dum_r = wp.tile([P, S], BF16, name="dum_r", tag="dum_r")
        nc.vector.memset(dum_r, 0.0)
        # two psum tiles from the scores pool (first real use ~19us, well
        # after the dummies drain) so consecutive dummies alternate banks
        # and pipeline at ~213ns instead of serializing on WAW drain.
        pdum = [
            ps.tile([P, 2 * S], F32, name=f"pdum{k}", tag="sc", bufs=2)
            for k in range(2)
        ]
        for k in range(30):
            nc.tensor.matmul(
                out=pdum[k % 2][0:1, :S], lhsT=dum_w[:, 0:1], rhs=dum_r,
                start=True, stop=True,
            )
        bcall = wp.tile([P, QKC], F32, name="bcall", tag="bcall")
        nc.gpsimd.dma_start(
            out=bcall, in_=bqkv[: 2 * D].rearrange("(c p) -> p c", p=P)
        )
        bcols = [bcall[:, c : c + 1] for c in range(QKC)]
        # bulk weight columns: chunk-pair consumption order, round-robin
        # across all three data queues.  The startup window is
        # DMA-bandwidth-bound (~300 GB/s aggregate, ~6 MB needed), so
        # column groups must arrive in the order the prologue consumes
        # them: q/k cols for chunk pairs (1,7),(2,8) first, then the v
        # columns (prologue v tiles), then pairs (3,9),(4,10),(5,11).
        qk_groups = [(P, 3 * P), (7 * P, 9 * P)]
        qk_groups_late = [
            (3 * P, 5 * P), (9 * P, 11 * P),
            (5 * P, 6 * P), (11 * P, 12 * P),
        ]
        qi = 0
        for lo, hi in qk_groups:
            for d in range(DCH):
                x_engs[qi % 3].dma_start(
                    out=wq_t[d][:, lo:hi], in_=wqkvT[d * P : (d + 1) * P, lo:hi]
                )
                qi += 1
        for d in range(DCH):
            x_engs[qi % 3].dma_start(
                out=wq_t[d][:, 2 * D :], in_=wqkvT[d * P : (d + 1) * P, 2 * D :]
            )
            qi += 1
        for lo, hi in qk_groups_late:
            for d in range(DCH):
                x_engs[qi % 3].dma_start(
                    out=wq_t[d][:, lo:hi], in_=wqkvT[d * P : (d + 1) * P, lo:hi]
                )
                qi += 1
        bvrow = wp.tile([1, D], BF16, name="bvrow", tag="bvrow")
        nc.gpsimd.dma_start(out=bvrow, in_=bv16.rearrange("(o f) -> o f", o=1))
        bvb = wp.tile([P, D], BF16, name="bvb", tag="bvb")
        nc.gpsimd.partition_broadcast(bvb, bvrow)
        ones = wp.tile([1, P], BF16, name="ones", tag="ones")
        nc.vector.memset(ones, 1.0)
        wp_t = []
        for d in range(DCH):
            t = wp.tile([P, D], BF16, name=f"wpT{d}", tag=f"wpT{d}")
            x_engs[d % 3].dma_start(out=t, in_=wpT[d * P : (d + 1) * P, :])
            wp_t.append(t)
        comborow = wp.tile([1, D], BF16, name="comborow", tag="comborow")
        nc.gpsimd.dma_start(out=comborow, in_=combo.rearrange("(o f) -> o f", o=1))
        cbb = wp.tile([P, D], BF16, name="cbb", tag="cbb")
        nc.gpsimd.partition_broadcast(cbb, comborow)

        def emit_x_load(b):
            xt = []
            for d in range(DCH):
                t = sb.tile([P, S], BF16, name=f"xT_b{b}_{d}", tag=f"xT{d}", bufs=2)
                x_engs[d % 3].dma_start(out=t, in_=xT[b, d * P : (d + 1) * P, :])
                xt.append(t)
            return xt

        def emit_yT_chunk(b, xt, c):
            pt = ps.tile([P, S], F32, name=f"yTps_b{b}_{c}", tag="yv", bufs=2)
            for d in range(DCH):
                nc.tensor.matmul(
                    out=pt,
                    lhsT=wq_t[d][:, c * P : (c + 1) * P],
                    rhs=xt[d],
                    start=(d == 0),
                    stop=(d == DCH - 1),
                )
            st = sb.tile([P, S], BF16, name=f"yT_b{b}_{c}", tag=f"yT{c}", bufs=2)
            nc.scalar.activation(
                st, pt, mybir.ActivationFunctionType.Identity, bias=bcols[c]
            )
            return st

        def emit_v_tile(b, xt, ti):
            vtile = sb.tile(
                [P, H * (HD + 1)], BF16, name=f"v_b{b}_{ti}", tag=f"v{ti}", bufs=2
            )
            nc.vector.memset(
                vtile.rearrange("p (h k) -> p h k", k=HD + 1)[:, :, HD : HD + 1],
                1.0,
            )
            for half in range(2):
                pv = ps.tile(
                    [P, NHALF], F32, name=f"vps_b{b}_{ti}_{half}", tag="av", bufs=2
                )
                o0 = 2 * D + half * NHALF
                for d in range(DCH):
                    nc.tensor.matmul(
                        out=pv,
                        lhsT=xt[d][:, ti * P : (ti + 1) * P],
                        rhs=wq_t[d][:, o0 : o0 + NHALF],
                        start=(d == 0),
                        stop=(d == DCH - 1),
                    )
                nc.vector.tensor_tensor(
                    out=vtile.rearrange("p (h k) -> p h k", k=HD + 1)[
                        :, 6 * half : 6 * (half + 1), 0:HD
                    ],
                    in0=pv.rearrange("p (h k) -> p h k", k=HD),
                    in1=bvb[:, half * NHALF : (half + 1) * NHALF].rearrange(
                        "p (h k) -> p h k", k=HD
                    ),
                    op=mybir.AluOpType.add,
                )
            return vtile

        def emit_scores_half(b, cp, yt, jp):
            """One kchunk-halfpair (kchunks 2jp,2jp+1) of scores for BOTH
            heads of chunk-pair cp.  Head 2cp lives on partitions 0:64,
            head 2cp+1 on 64:128, so the K=64 matmuls auto-derive row-tile
            positions (0,0)/(64,0) in 64x128 mode: interleaving A/B makes
            adjacent MMs land on disjoint row groups and run concurrently
            (~2x scores throughput vs the per-head serial order)."""
            qt, kt = yt[cp], yt[6 + cp]
            ets = []
            for jj in range(2):
                j = 2 * jp + jj
                # both heads' kchunk-j outputs share ONE psum tile
                # (disjoint banks): a single allocation event makes the
                # two MMs ready together with adjacent priorities, so the
                # scheduler co-issues them and the disjoint row groups
                # overlap in the array.
                pt = ps.tile(
                    [P, 2 * S], F32,
                    name=f"sc_b{b}_c{cp}_j{j}", tag="sc", bufs=2,
                )
                for hi, hp in enumerate((0, HD)):
                    nc.tensor.matmul(
                        out=pt[:, hi * S : (hi + 1) * S],
                        lhsT=kt[hp : hp + HD, j * P : (j + 1) * P],
                        rhs=qt[hp : hp + HD, :],
                        start=True,
                        stop=True,
                    )
                et = sb.tile(
                    [P, 2 * S], BF16,
                    name=f"expT_b{b}_c{cp}_j{j}", tag="expT", bufs=6,
                )
                nc.scalar.activation(et, pt, EXP, scale=SCALE)
                ets.append(et)
            return ets

        def emit_av(b, h, exps, vt, avt, state):
            # exps: 4 tiles (kchunks j=0..3), each [A_j | B_j]; this head's
            # queries sit in column-half h%2.
            hi = h % 2
            pav = ps.tile([HD + 1, S], F32, name=f"av_b{b}_h{h}", tag="av", bufs=2)
            for j in range(TCH):
                nc.tensor.matmul(
                    out=pav,
                    lhsT=vt[j][:, h * (HD + 1) : (h + 1) * (HD + 1)],
                    rhs=exps[j][:, hi * S : (hi + 1) * S],
                    start=(j == 0),
                    stop=(j == TCH - 1),
                )
            if b == NB - 1 and h >= 10:
                # tail fast path: the av psum has no later users, so skip
                # the copy/stacked hop — reciprocal on the psum sums row,
                # broadcast, multiply straight from psum.
                # per-head chain here (NOT paired): the final avt chunk
                # gates the last proj MMs, so chain LATENCY matters more
                # than gpsimd throughput at the tail.
                rr = sb.tile([1, S], F32, name=f"rr_l_{h}", tag="rrow",
                             bufs=6)
                _act_reciprocal(nc, rr, pav[HD : HD + 1, :])
                bc = sb.tile([HD, S], F32, name=f"bc_l_{h}", tag="bc",
                             bufs=3)
                nc.gpsimd.partition_broadcast(bc, rr)
                c = h // 2
                if h % 2 == 1:
                    nc.vector.tensor_mul(avt[c][:HD, :], pav[:HD, :], bc)
                else:
                    tmp = sb.tile([HD, S], BF16, name=f"avtmp_l_{h}",
                                  tag="avtmp", bufs=4)
                    nc.vector.tensor_mul(tmp, pav[:HD, :], bc)
                    nc.sync.dma_start(out=avt[c][HD : 2 * HD, :], in_=tmp)
                return
            # DVE copy frees the psum bank fast and gives the sums row a
            # DMA-able SBUF home; f32 so the wave reciprocal can use the
            # fast approximate path.
            avsb = sb.tile([HD + 1, S], F32, name=f"avsb_b{b}_h{h}", tag="avsb",
                           bufs=8)
            nc.vector.tensor_copy(avsb, pav)
            state["avsbs"].append(avsb)
            wi = h - state["wave_start"]
            nc.sync.dma_start(
                out=state["stacked"][32 * wi : 32 * wi + 4, :],
                in_=avsb[HD : HD + 1, :],
            )
            if h in state["wave_ends"]:
                w0 = state["wave_start"]
                recw = sb.tile([P, P], F32, name=f"recw_b{b}_h{h}",
                               tag="recw", bufs=2)
                nc.vector.reciprocal_approx_fast(recw, state["stacked"])
                hhs = list(range(w0, h + 1))
                if len(hhs) == 2 and hhs[0] % 2 == 0:
                    # head-pair wave: gather both heads' reciprocal rows
                    # into one [1,2S] tile and do a SINGLE partition
                    # broadcast (the bcast has ~0.8us fixed overhead, so
                    # pairing nearly halves the gpsimd chain latency).
                    # Odd head's multiply lands in avT directly; even head
                    # goes via a tmp that a DMA partition-shifts.
                    hA, hB = hhs
                    c = hB // 2
                    rrp = sb.tile([1, 2 * S], F32, name=f"rrp_b{b}_h{hB}",
                                  tag="rrow", bufs=6)
                    nc.gpsimd.dma_start(out=rrp[:, :S], in_=recw[0:4, :])
                    nc.gpsimd.dma_start(out=rrp[:, S:], in_=recw[32:36, :])
                    bcp = sb.tile([HD, 2 * S], F32, name=f"bcp_b{b}_h{hB}",
                                  tag="bc", bufs=3)
                    nc.gpsimd.partition_broadcast(bcp, rrp)
                    nc.vector.tensor_mul(
                        avt[c][:HD, :], state["avsbs"][hB][:HD, :], bcp[:, S:]
                    )
                    tmp = sb.tile([HD, S], BF16, name=f"avtmp_b{b}_h{hA}",
                                  tag="avtmp", bufs=4)
                    nc.vector.tensor_mul(
                        tmp, state["avsbs"][hA][:HD, :], bcp[:, :S]
                    )
                    nc.sync.dma_start(out=avt[c][HD : 2 * HD, :], in_=tmp)
                else:
                    for hh in sorted(hhs, key=lambda x: x % 2 == 0):
                        wj = hh - w0
                        rrow = sb.tile([1, S], F32, name=f"rrow_b{b}_h{hh}",
                                       tag="rrow", bufs=6)
                        nc.gpsimd.dma_start(
                            out=rrow, in_=recw[32 * wj : 32 * wj + 4, :]
                        )
                        bc = sb.tile([HD, S], F32, name=f"bc_b{b}_h{hh}",
                                     tag="bc", bufs=3)
                        nc.gpsimd.partition_broadcast(bc, rrow)
                        c = hh // 2
                        src_av = state["avsbs"][hh]
                        if hh % 2 == 1:
                            nc.vector.tensor_mul(
                                avt[c][:HD, :], src_av[:HD, :], bc
                            )
                        else:
                            tmp = sb.tile([HD, S], BF16,
                                          name=f"avtmp_b{b}_h{hh}",
                                          tag="avtmp", bufs=4)
                            nc.vector.tensor_mul(tmp, src_av[:HD, :], bc)
                            nc.sync.dma_start(
                                out=avt[c][HD : 2 * HD, :], in_=tmp
                            )
                state["stacked"] = sb.tile(
                    [P, P], F32, name=f"stk_b{b}_h{h}", tag="stacked", bufs=2
                )
                nc.vector.memset(state["stacked"], 1.0)
                state["wave_start"] = h + 1

        def emit_proj_group(b, avt, gi):
            ti, half = gi // 2, gi % 2
            pf = ps.tile(
                [P, NHALF], F32, name=f"fps_b{b}_{ti}_{half}", tag="yv", bufs=2
            )
            for d in range(DCH):
                nc.tensor.matmul(
                    out=pf,
                    lhsT=avt[d][:, ti * P : (ti + 1) * P],
                    rhs=wp_t[d][:, half * NHALF : (half + 1) * NHALF],
                    start=(d == 0),
                    stop=(d == DCH - 1),
                )
            ft = sb.tile(
                [P, NHALF], BF16, name=f"fin_b{b}_{gi}", tag="fin", bufs=4
            )
            nc.vector.tensor_tensor(
                out=ft,
                in0=pf,
                in1=cbb[:, half * NHALF : (half + 1) * NHALF],
                op=mybir.AluOpType.add,
            )
            (nc.sync if half == 0 else nc.scalar).dma_start(
                out=out[
                    b, ti * P : (ti + 1) * P, half * NHALF : (half + 1) * NHALF
                ],
                in_=ft,
            )

        # ---- main schedule: flat pipeline.  Batch 0's qkv runs as a
        # prologue; afterwards every head-slot of attn(b) carries one yT
        # chunk of b+1, one proj group of b-1 (slots 1..8), and a v tile
        # of b+1 (slots 8..11), so the scalar engine's exp stream always
        # has matmul work to hide behind.  x loads lag two batches. ----
        CHUNK_ORDER = [0, 6, 1, 7, 2, 8, 3, 9, 4, 10, 5, 11]
        xt_next = emit_x_load(1)  # x(1); x(0) loaded above
        yt_cur = [None] * QKC
        # first chunk pair with split accumulation so the PE starts as
        # soon as the first three x / weight-column DMAs land
        pts0 = {}
        for c in (0, 6):
            pt = ps.tile([P, S], F32, name=f"yTps_b0_{c}", tag="yv", bufs=2)
            for d in range(3):
                nc.tensor.matmul(
                    out=pt, lhsT=wq_t[d][:, c * P : (c + 1) * P], rhs=xt0[d],
                    start=(d == 0), stop=False,
                )
            pts0[c] = pt
        for c in (0, 6):
            pt = pts0[c]
            for d in range(3, DCH):
                nc.tensor.matmul(
                    out=pt, lhsT=wq_t[d][:, c * P : (c + 1) * P], rhs=xt0[d],
                    start=False, stop=(d == DCH - 1),
                )
            st = sb.tile([P, S], BF16, name=f"yT_b0_{c}", tag=f"yT{c}", bufs=2)
            nc.scalar.activation(
                st, pt, mybir.ActivationFunctionType.Identity, bias=bcols[c]
            )
            yt_cur[c] = st
        # prologue order matches DMA arrival: chunk pairs (1,7),(2,8),
        # then the v tiles (their weight columns land before the late
        # q/k column groups), then (3,9),(4,10),(5,11).
        for hp in (1, 2):
            for c in (hp, 6 + hp):
                yt_cur[c] = emit_yT_chunk(0, xt0, c)
        vt_cur = [emit_v_tile(0, xt0, ti) for ti in range(TCH)]
        for hp in (3, 4, 5):
            for c in (hp, 6 + hp):
                yt_cur[c] = emit_yT_chunk(0, xt0, c)
        prev = None
        xt_n2 = None
        for b in range(NB):
            avt = [
                sb.tile([P, S], BF16, name=f"avT_b{b}_{c}", tag=f"avT{c}", bufs=2)
                for c in range(DCH)
            ]
            state = {
                "avsbs": [],
                "wave_start": 0,
                "wave_ends": (
                    {1, 3, 5, 7, 8, 9} if b == NB - 1 else {1, 3, 5, 7, 9, 11}
                ),
                "stacked": sb.tile([P, P], F32, name=f"stk_b{b}_init",
                                   tag="stacked", bufs=2),
            }
            nc.vector.memset(state["stacked"], 1.0)
            yt_next = [None] * QKC
            vt_next = [None] * TCH
            pexp = None  # (halfpair jp0, halfpair jp1) of previous pair
            for cp in range(H // 2):
                # b=0: x(1) lands ~20us after the prologue starts chewing,
                # so push b=1's qkv chunks one slot later — the PE queue is
                # in-order and a not-yet-ready chunk MM would stall the
                # whole stream behind it.
                if b + 1 >= NB:
                    slot_chunks = []
                elif b == 0:
                    slot_chunks = (
                        [] if cp == 0
                        else CHUNK_ORDER[8:] if cp == 5
                        else CHUNK_ORDER[2 * (cp - 1) : 2 * cp]
                    )
                else:
                    slot_chunks = CHUNK_ORDER[2 * cp : 2 * cp + 2]
                half = (len(slot_chunks) + 1) // 2
                e0 = emit_scores_half(b, cp, yt_cur, 0)
                for c in slot_chunks[:half]:
                    yt_next[c] = emit_yT_chunk(b + 1, xt_next, c)
                if prev is not None and 1 <= cp <= 4:
                    emit_proj_group(b - 1, prev, 2 * (cp - 1))
                e1 = emit_scores_half(b, cp, yt_cur, 1)
                for c in slot_chunks[half:]:
                    yt_next[c] = emit_yT_chunk(b + 1, xt_next, c)
                if prev is not None and 1 <= cp <= 4:
                    emit_proj_group(b - 1, prev, 2 * cp - 1)
                if cp > 0:
                    emit_av(b, 2 * (cp - 1), pexp, vt_cur, avt, state)
                    emit_av(b, 2 * cp - 1, pexp, vt_cur, avt, state)
                pexp = e0 + e1
                if b + 1 < NB and cp >= 4:
                    ti = 2 * (cp - 4)
                    vt_next[ti] = emit_v_tile(b + 1, xt_next, ti)
                    vt_next[ti + 1] = emit_v_tile(b + 1, xt_next, ti + 1)
                if cp == 1 and b + 2 < NB:
                    xt_n2 = emit_x_load(b + 2)
            emit_av(b, H - 2, pexp, vt_cur, avt, state)
            emit_av(b, H - 1, pexp, vt_cur, avt, state)
            prev = avt
            yt_cur, vt_cur = yt_next, vt_next
            xt_next, xt_n2 = xt_n2, None
        # final batch's proj: six concurrent groups accumulate d=0..4
        # while the last normalization wave drains, then take d=5; the
        # remaining two output chunks follow.
        tags6 = ["yv", "yv", "sc", "sc", "av", "av"]
        groups = []
        for gi in range(6):
            ti, half = gi // 2, gi % 2
            pf = ps.tile(
                [P, NHALF], F32, name=f"fps_l_{ti}_{half}", tag=tags6[gi],
                bufs=2,
            )
            groups.append((pf, ti, half, gi))
        for pf, ti, half, gi in groups:
            for d in range(5):
                nc.tensor.matmul(
                    out=pf,
                    lhsT=prev[d][:, ti * P : (ti + 1) * P],
                    rhs=wp_t[d][:, half * NHALF : (half + 1) * NHALF],
                    start=(d == 0),
                    stop=False,
                )
        for pf, ti, half, gi in groups:
            nc.tensor.matmul(
                out=pf,
                lhsT=prev[5][:, ti * P : (ti + 1) * P],
                rhs=wp_t[5][:, half * NHALF : (half + 1) * NHALF],
                start=False,
                stop=True,
            )
            ft = sb.tile(
                [P, NHALF], BF16, name=f"fin_l_{gi}", tag="fin", bufs=4
            )
            nc.vector.tensor_tensor(
                out=ft,
                in0=pf,
                in1=cbb[:, half * NHALF : (half + 1) * NHALF],
                op=mybir.AluOpType.add,
            )
            (nc.sync if half == 0 else nc.scalar).dma_start(
                out=out[
                    NB - 1,
                    ti * P : (ti + 1) * P,
                    half * NHALF : (half + 1) * NHALF,
                ],
                in_=ft,
            )
        for gi in (6, 7):
            emit_proj_group(NB - 1, prev, gi)

    nc.compile()
    return nc


_CACHE = {}


def _get_nc():
    if "nc" not in _CACHE:
        _CACHE["nc"] = build_nc()
    return _CACHE["nc"]


def _prepare_in_maps(x, qkv_w, qkv_b, proj_w, proj_b):
    x = np.asarray(x, dtype=np.float32)
    qkv_w = np.asarray(qkv_w, dtype=np.float32)
    qkv_b = np.asarray(qkv_b, dtype=np.float32)
    proj_w = np.asarray(proj_w, dtype=np.float32)
    proj_b = np.asarray(proj_b, dtype=np.float32)
    import ml_dtypes

    bf16 = ml_dtypes.bfloat16
    wqkvT = np.ascontiguousarray(qkv_w.T).astype(bf16)
    wpT = np.ascontiguousarray(proj_w.T)
    # chunk c of wpT rows feeds avT[c] = [head 2c+1 (rows 0:64); head 2c]
    wpT = np.ascontiguousarray(
        wpT.reshape(6, 2, 64, D)[:, ::-1].reshape(D, D)
    ).astype(bf16)
    combo = proj_b.astype(bf16)  # v-bias flows through softmax via bvrow
    bv16 = qkv_b[2 * D :].astype(bf16)
    in_maps = []
    for c in range(NCORES):
        xs = x[c * NB : (c + 1) * NB]
        xTs = np.ascontiguousarray(xs.transpose(0, 2, 1)).astype(bf16)
        in_maps.append(
            {
                "xT": xTs,
                "wqkvT": wqkvT,
                "wpT": wpT,
                "bqkv": qkv_b,
                "combo": combo,
                "bv16": bv16,
            }
        )
    return in_maps


def kernel(x, qkv_w, qkv_b, proj_w, proj_b):
    nc = _get_nc()
    in_maps = _prepare_in_maps(x, qkv_w, qkv_b, proj_w, proj_b)
    res = run_bass_kernel_spmd(nc, in_maps, core_ids=list(range(NCORES)))
    return np.concatenate(
        [res.results[i]["out"].astype(np.float32) for i in range(NCORES)], axis=0
    )

